# revision 2
# baseline (speedup 1.0000x reference)
"""AnomalyMapGenerator (retrieval kNN) Trainium2 kernel.

reference:  d = sqrt(distance[B, HW, M]); v = 3 smallest of d per row;
            w = softmax(-v); s = w0*v0 -> [B, 56, 56]
            -> bilinear resize to 224x224 -> gaussian blur (sigma=4, reflect).

Strategy (8 NeuronCores, data-parallel over batch, 2 images per core):
  - per core, rows r = b*3136 + hw (6272 rows of 4096 distances).
    Row->SBUF mapping r = 49*p + t: tile t holds rows {49p+t}, so the
    per-row scalar s lands in SBUF as [128, 49] in linear row order.
  - main loop (49 tiles of [128, 4096] f32, 2 MiB each):
      DMA load -> ScalarE negate -> VectorE max8 (top-8 of -d = 3 smallest of d,
      duplicate multiplicity preserved, matching lax.top_k).
  - tail: v = sqrt(-top3) (ScalarE), E = exp(-v) (ScalarE),
      s = v0*E0 / (E0+E1+E2) (VectorE), DMA s -> DRAM smap [6272].
  - post: resize+blur are one linear operator A = G_blur @ R_resize [224, 56];
      per image: out = A @ S @ A^T via two TensorE matmuls with
      amat_t = A^T [56, 224] (both stages use the same operand, no transposes).
"""
import os
import numpy as np

B, HW, M = 16, 3136, 4096
IMG_IN, IMG_OUT, SIGMA = 56, 224, 4.0
N_CORES = 8
BPC = B // N_CORES            # images per core
ROWS = BPC * HW               # 6272
P = 128
T = ROWS // P                 # 49 tiles
HALF = IMG_OUT // 2           # 112

_CACHE = {}


def _resize_mat(in_size: int, out_size: int) -> np.ndarray:
    # jax.image.resize(method='bilinear') upsampling weight matrix [out, in]
    scale = out_size / in_size
    sample_f = (np.arange(out_size, dtype=np.float64) + 0.5) / scale - 0.5
    x = np.abs(sample_f[None, :] - np.arange(in_size, dtype=np.float64)[:, None])
    w = np.maximum(0.0, 1.0 - x)
    total = w.sum(axis=0, keepdims=True)
    w = np.where(np.abs(total) > 1e-8, w / total, 0.0)
    ob = (sample_f < -0.5) | (sample_f > in_size - 0.5)
    w[:, ob] = 0.0
    return w.T


def _gauss_mat(n: int, sigma: float) -> np.ndarray:
    # 1D gaussian conv with reflect padding as a matrix [n, n]
    ksize = 2 * int(4.0 * sigma + 0.5) + 1
    xs = np.arange(ksize, dtype=np.float64) - ksize // 2
    g = np.exp(-(xs * xs) / (2.0 * sigma * sigma))
    g = g / g.sum()
    pad = ksize // 2
    Gm = np.zeros((n, n), dtype=np.float64)
    for o in range(n):
        for k in range(ksize):
            idx = o - pad + k
            if idx < 0:
                idx = -idx
            elif idx > n - 1:
                idx = 2 * (n - 1) - idx
            Gm[o, idx] += g[k]
    return Gm


def _amat_t() -> np.ndarray:
    A = _gauss_mat(IMG_OUT, SIGMA) @ _resize_mat(IMG_IN, IMG_OUT)  # [224, 56]
    return np.ascontiguousarray(A.T.astype(np.float32))            # [56, 224]


def _build():
    from contextlib import ExitStack
    import concourse.bass as bass
    import concourse.tile as tile
    from concourse import bacc, mybir

    f32 = mybir.dt.float32
    AF = mybir.ActivationFunctionType

    nc = bacc.Bacc("TRN2", target_bir_lowering=False, debug=False,
                   enable_asserts=False)
    dist = nc.dram_tensor("distance", [ROWS, M], f32, kind="ExternalInput")
    amat = nc.dram_tensor("amat_t", [IMG_IN, IMG_OUT], f32, kind="ExternalInput")
    out = nc.dram_tensor("out", [BPC, IMG_OUT, IMG_OUT], f32, kind="ExternalOutput")
    smap = nc.dram_tensor("smap", [ROWS], f32)  # internal scratch

    distv = dist.ap().rearrange("(p t) m -> p t m", p=P)      # r = 49p + t
    smap_pt = smap.ap().rearrange("(p t) -> p t", p=P)
    smap_img = smap.ap().rearrange("(i h w) -> i h w", i=BPC, h=IMG_IN)
    out_ap = out.ap()

    with tile.TileContext(nc) as tc, ExitStack() as ctx:
        pool_in = ctx.enter_context(tc.tile_pool(name="in", bufs=4))
        pool_neg = ctx.enter_context(tc.tile_pool(name="neg", bufs=3))
        pool_keep = ctx.enter_context(tc.tile_pool(name="keep", bufs=1))
        pool_mm = ctx.enter_context(tc.tile_pool(name="mm", bufs=2))
        pool_ps = ctx.enter_context(
            tc.tile_pool(name="ps", bufs=2, space="PSUM"))

        amat_sb = pool_keep.tile([IMG_IN, IMG_OUT], f32)
        nc.sync.dma_start(amat_sb[:], amat.ap())

        top8 = pool_keep.tile([P, 8 * T], f32)
        for t in range(T):
            tin = pool_in.tile([P, M], f32)
            nc.sync.dma_start(tin[:], distv[:, t, :])
            tneg = pool_neg.tile([P, M], f32)
            nc.scalar.mul(tneg[:], tin[:], -1.0)
            nc.vector.max(top8[:, 8 * t:8 * t + 8], tneg[:])

        # tail: softmin-weighted minimum per row
        top8v = top8[:].rearrange("p (t e) -> p e t", e=8)
        vall = pool_keep.tile([P, 3 * T], f32)   # [v0 | v1 | v2] blocks
        for e in range(3):
            nc.scalar.activation(vall[:, e * T:(e + 1) * T], top8v[:, e, :],
                                 AF.Sqrt, scale=-1.0)
        eall = pool_keep.tile([P, 3 * T], f32)
        nc.scalar.activation(eall[:], vall[:], AF.Exp, scale=-1.0)
        denom = pool_keep.tile([P, T], f32)
        nc.vector.tensor_add(denom[:], eall[:, 0:T], eall[:, T:2 * T])
        nc.vector.tensor_add(denom[:], denom[:], eall[:, 2 * T:3 * T])
        rec = pool_keep.tile([P, T], f32)
        nc.vector.reciprocal(rec[:], denom[:])
        sval = pool_keep.tile([P, T], f32)
        nc.vector.tensor_mul(sval[:], vall[:, 0:T], eall[:, 0:T])
        nc.vector.tensor_mul(sval[:], sval[:], rec[:])
        nc.sync.dma_start(smap_pt, sval[:])

        # post: out_i = A @ S_i @ A^T
        for i in range(BPC):
            s_i = pool_mm.tile([IMG_IN, IMG_IN], f32)
            nc.sync.dma_start(s_i[:], smap_img[i, :, :])
            ps1 = pool_ps.tile([IMG_IN, IMG_OUT], f32)
            # ps1[w', n] = sum_h S[h, w'] * A[n, h]  ==  (A @ S)^T
            nc.tensor.matmul(ps1[:], s_i[:], amat_sb[:], start=True, stop=True)
            u1 = pool_mm.tile([IMG_IN, IMG_OUT], f32)
            nc.scalar.copy(u1[:], ps1[:])
            for c in range(2):
                ps2 = pool_ps.tile([HALF, IMG_OUT], f32)
                # ps2[ho, w] = sum_w' (A@S)[ho, w'] * A[w, w']
                nc.tensor.matmul(ps2[:], u1[:, c * HALF:(c + 1) * HALF],
                                 amat_sb[:], start=True, stop=True)
                o_c = pool_mm.tile([HALF, IMG_OUT], f32)
                nc.scalar.copy(o_c[:], ps2[:])
                nc.sync.dma_start(out_ap[i, c * HALF:(c + 1) * HALF, :], o_c[:])

    nc.compile()
    return nc


def _get_nc():
    if "nc" not in _CACHE:
        _CACHE["nc"] = _build()
    return _CACHE["nc"]


def kernel(**inputs) -> np.ndarray:
    from concourse.bass_utils import run_bass_kernel_spmd

    distance = np.ascontiguousarray(np.asarray(inputs["distance"], dtype=np.float32))
    assert distance.shape == (B, HW, M), distance.shape
    amat_t = _amat_t()

    nc = _get_nc()
    in_maps = []
    for c in range(N_CORES):
        shard = distance[c * BPC:(c + 1) * BPC].reshape(ROWS, M)
        in_maps.append({"distance": shard, "amat_t": amat_t})

    trace = bool(int(os.environ.get("KERNEL_TRACE", "0")))
    try:
        res = run_bass_kernel_spmd(nc, in_maps, core_ids=list(range(N_CORES)),
                                   trace=trace)
    except ModuleNotFoundError:
        if not trace:
            raise
        trace = False
        res = run_bass_kernel_spmd(nc, in_maps, core_ids=list(range(N_CORES)),
                                   trace=False)
    if trace and res.exec_time_ns is not None:
        print(f"HW exec time: {res.exec_time_ns} ns")
        _CACHE["exec_time_ns"] = res.exec_time_ns
        _CACHE["results"] = res

    outs = [res.results[c]["out"] for c in range(N_CORES)]
    full = np.concatenate(outs, axis=0).reshape(B, 1, IMG_OUT, IMG_OUT)
    return full.astype(np.float32)
